# revision 67
# baseline (speedup 1.0000x reference)
"""KNRM kernel for 8 Trainium2 NeuronCores (data-parallel over batch).

Per core (32 batches):
  - host: dedup this core's tokens (~16k unique < int16 max), build a
    pre-normalized bf16 mini-table [17472, 384] (300 emb dims + mask-bias
    column at 300: -1e6 for vocab id 0, else 0), remap token tensors to
    int16 mini-table indices replicated across 16-partition groups (the
    Q7 dma_gather ucode reads a copy per 16-partition channel group).
  - device: per 2048-token chunk, ONE dma_gather(transpose=True) delivers
    embeddings directly in [e, token] layout (partition p, free slot j
    holds element 128j+p), so the cosine matmuls need no PE transposes and
    no PSUM->SBUF copies. Masking is folded into the contraction via the
    bias column (query side forced to 1.0). Gaussian kernel pooling runs
    as ONE scalar-engine pass per kernel using Derivative_Erf
    (d/dx erf = 2/sqrt(pi) * exp(-x^2)) with free-dim accumulation,
    reading cos straight from PSUM; the 2/sqrt(pi) factor is undone by
    the Ln(scale=sqrt(pi)/2) in the tail. k0 (sigma=1e-4, exact token
    match) is a DVE token-equality count scaled by 2/sqrt(pi) to share
    the same tail.
"""

import sys

sys.path.insert(0, "/opt/trn_rl_repo")

import numpy as np

B, Q, D, V, E = 256, 20, 512, 100000, 300
NCORES = 8
BLOC = B // NCORES  # 32 batches per core
QPAD = 32  # query slots per batch (20 real + 12 pad)
NQTOK = BLOC * QPAD  # 1024 query gather slots per core
DCHUNKS = 8  # doc chunks per core
DCTOK = 2048  # doc tokens per chunk (= 4 batches)
NK = 11
MSLOT = 384  # mini-table row elems (bf16) -> 768B, 256B-multiple
MROWS = 17472  # >= max unique tokens per core (16384 doc + 1024 q)
BIAS_COL = 300
MASK_BIAS = -1.0e6

SQRT50 = float(np.sqrt(50.0))
LN_SCALE = float(np.sqrt(np.pi) / 2.0)  # undo derf's 2/sqrt(pi)
K0_SCALE = float(2.0 / np.sqrt(np.pi))
CLIP = 1e-10 / LN_SCALE
# Ln(LN_SCALE*x) applied to the raw k0 count leaves an extra ln(LN_SCALE);
# the correction (-ln(LN_SCALE) per valid q row) is folded into the dense
# bias on the host
LNC = float(-np.log(LN_SCALE))


def _mus(n):
    l = [1.0]
    bs = 2.0 / (n - 1)
    l.append(1 - bs / 2)
    for i in range(1, n - 1):
        l.append(l[i] - bs)
    return l


MUS = _mus(NK)

_prog_cache = {}
DEBUG = False


def _build_program():
    key = ("nc", DEBUG)
    if key in _prog_cache:
        return _prog_cache[key]

    import concourse.bass as bass
    import concourse.bacc as bacc
    import concourse.mybir as mybir
    import concourse.tile as tile
    from concourse import library_config

    f32 = mybir.dt.float32
    bf16 = mybir.dt.bfloat16
    i16 = mybir.dt.int16
    AF = mybir.ActivationFunctionType
    ALU = mybir.AluOpType

    nc = bacc.Bacc(
        "TRN2",
        target_bir_lowering=False,
        debug=False,
        num_devices=NCORES,
        num_swdge_queues=4,
    )

    mtab = nc.dram_tensor("mtab", [MROWS, MSLOT], bf16, kind="ExternalInput").ap()
    d_idx = nc.dram_tensor(
        "d_idx", [128, DCHUNKS * 4 * (512 // 16)], i16, kind="ExternalInput"
    ).ap()
    q_idx = nc.dram_tensor(
        "q_idx", [2, 128, 512 // 16], i16, kind="ExternalInput"
    ).ap()
    s_sel = nc.dram_tensor("s_sel", [128, 4], f32, kind="ExternalInput").ap()
    d_tokf = nc.dram_tensor(
        "d_tokf", [128, DCHUNKS * 512], f32, kind="ExternalInput"
    ).ap()
    q_tokf = nc.dram_tensor("q_tokf", [128, DCHUNKS], f32, kind="ExternalInput").ap()
    qm88 = nc.dram_tensor(
        "qm88", [128, DCHUNKS * NK], f32, kind="ExternalInput"
    ).ap()
    w88 = nc.dram_tensor("w88", [4, DCHUNKS * NK], f32, kind="ExternalInput").ap()
    beff = nc.dram_tensor("beff", [4, DCHUNKS], f32, kind="ExternalInput").ap()

    derfb = nc.dram_tensor("derfb", [128, NK], f32, kind="ExternalInput").ap()
    qones = nc.dram_tensor("qones", [1, NQTOK], bf16, kind="ExternalInput").ap()
    out = nc.dram_tensor("out", [4, DCHUNKS], f32, kind="ExternalOutput").ap()
    dbg_pkq = (
        nc.dram_tensor("dbg_pkq", [DCHUNKS, 128, NK], f32, kind="ExternalOutput").ap()
        if DEBUG
        else None
    )
    dbg_cos = (
        nc.dram_tensor("dbg_cos", [DCHUNKS, 128, 512], f32, kind="ExternalOutput").ap()
        if DEBUG
        else None
    )

    with tile.TileContext(nc) as tc:
        import contextlib

        with contextlib.ExitStack() as ctx:
            const_pool = ctx.enter_context(tc.tile_pool(name="consts", bufs=1))
            qp = ctx.enter_context(tc.tile_pool(name="qprep", bufs=1))
            dpool = ctx.enter_context(tc.tile_pool(name="demb", bufs=3))
            pkpool = ctx.enter_context(tc.tile_pool(name="pk", bufs=1))
            scr = ctx.enter_context(tc.tile_pool(name="scr", bufs=2))
            psum = ctx.enter_context(
                tc.tile_pool(name="psum", bufs=2, space="PSUM")
            )
            psum1 = ctx.enter_context(
                tc.tile_pool(name="psum1", bufs=1, space="PSUM")
            )

            nc.gpsimd.load_library(library_config.mlp)

            # ---------------- Q gather (transposed), 2x512 idxs ----------------
            # queue_num must track the global SWDGE issue index mod 4 so that
            # Tile's round-robin DMA-sem lanes (mod 8) never mix queues: a
            # lane shared by two queues completes out of issue order and the
            # lane's wait threshold passes early (observed as stale dT reads)
            gather_i = 0
            qT = qp.tile([128, 2 * 3 * 512], bf16)
            qT4 = qT[:].rearrange("p (g j n) -> p g j n", g=2, j=3)
            for g in range(2):
                qi = qp.tile([128, 512 // 16], i16, tag=f"qi{g}", name=f"qi{g}")
                nc.sync.dma_start(out=qi[:], in_=q_idx[g])
                nc.gpsimd.dma_gather(
                    out_ap=qT4[:, g],
                    in_ap=mtab[:],
                    idxs_ap=qi[:],
                    num_idxs=512,
                    num_idxs_reg=512,
                    elem_size=MSLOT,
                    transpose=True,
                    queue_num=gather_i % 4,
                )
                gather_i += 1
                # query-side bias multiplier: force e-row 300 (tile 2, part 44)
                nc.sync.dma_start(
                    out=qT4[44:45, g : g + 1, 2, :], in_=qones[:, 512 * g : 512 * (g + 1)]
                )

            # all doc-gather indices in one upfront DMA (host pre-arranged)
            di_all = qp.tile([128, DCHUNKS * 4 * (512 // 16)], i16)
            di4 = di_all[:].rearrange("p (h b n) -> p h b n", h=DCHUNKS, b=4)
            nc.sync.dma_start(out=di_all[:], in_=d_idx[:])

            # k0-compare inputs next (the compares run early, off-path, on DVE)
            qtokf_t = const_pool.tile([128, DCHUNKS], f32)
            nc.sync.dma_start(out=qtokf_t[:], in_=q_tokf[:])
            # doc tokens pre-broadcast to all 128 partitions on the host, so
            # the k0 equality check needs no PE broadcast matmul
            dtf_all = qp.tile([128, DCHUNKS * 512], f32)
            dtf3 = dtf_all[:].rearrange("p (h n) -> p h n", h=DCHUNKS)
            nc.sync.dma_start(out=dtf_all[:], in_=d_tokf[:])
            # tail-only consts last
            s_sel_t = const_pool.tile([128, 4], f32)
            nc.sync.dma_start(out=s_sel_t[:], in_=s_sel[:])
            w88_t = const_pool.tile([4, DCHUNKS * NK], f32)
            nc.sync.dma_start(out=w88_t[:], in_=w88[:])
            beff_t = const_pool.tile([4, DCHUNKS], f32)
            nc.sync.dma_start(out=beff_t[:], in_=beff[:])
            derfb_t = const_pool.tile([128, NK], f32)
            nc.sync.dma_start(out=derfb_t[:], in_=derfb[:])
            qm88_t = const_pool.tile([128, DCHUNKS * NK], f32)
            nc.sync.dma_start(out=qm88_t[:], in_=qm88[:])

            # ---------------- main loop over chunk pairs ----------------
            # all chunks' pooled sums live in one [128, 8*11] tile: chunk h
            # owns columns 11h..11h+11 (k0 at 11h)
            pkq_all = pkpool.tile([128, DCHUNKS * NK], f32, tag="pkqall")
            pk3 = pkq_all[:].rearrange("p (h k) -> p h k", k=NK)
            groups = [(0, 1, 2), (3, 4, 5), (6, 7)]
            for grp in groups:
                glen = len(grp)
                # the group's chunks share one [128, 512*glen] PSUM cos tile
                # so each derf pass covers the whole group; the per-chunk sums
                # come from a segmented DVE reduce afterwards
                cosfull = psum.tile([128, 512 * 3], f32, tag="cos")
                cos = cosfull[:, : 512 * glen]
                for j, h in enumerate(grp):
                    dT = dpool.tile([128, 4 * 3 * 512], bf16, tag="demb")
                    dT4 = dT[:].rearrange("p (b j n) -> p b j n", b=4, j=3)
                    for beta in range(4):
                        nc.gpsimd.dma_gather(
                            out_ap=dT4[:, beta],
                            in_ap=mtab[:],
                            idxs_ap=di4[:, h, beta],
                            num_idxs=512,
                            num_idxs_reg=512,
                            elem_size=MSLOT,
                            transpose=True,
                            queue_num=gather_i % 4,
                        )
                        gather_i += 1



                    for beta in range(4):
                        b_glob = 4 * h + beta
                        g, qs = b_glob // 16, QPAD * (b_glob % 16)
                        cob = cos[32 * beta : 32 * beta + 32, 512 * j : 512 * (j + 1)]
                        nc.tensor.matmul(
                            out=cob,
                            lhsT=qT4[:, g, 0, qs : qs + QPAD],
                            rhs=dT4[:, beta, 0, :],
                            start=True,
                            stop=False,
                            tile_position=(0, 32 * beta),
                        )
                        nc.tensor.matmul(
                            out=cob,
                            lhsT=qT4[:, g, 1, qs : qs + QPAD],
                            rhs=dT4[:, beta, 1, :],
                            start=False,
                            stop=False,
                            tile_position=(0, 32 * beta),
                        )
                        nc.tensor.matmul(
                            out=cob,
                            lhsT=qT4[0:45, g, 2, qs : qs + QPAD],
                            rhs=dT4[0:45, beta, 2, :],
                            start=False,
                            stop=True,
                            tile_position=(0, 32 * beta),
                        )

                    # k0: exact-token-match count
                    cmp = scr.tile([128, 512], f32, tag="cmp")
                    nc.vector.tensor_scalar(
                        out=cmp[:],
                        in0=dtf3[:, h],
                        scalar1=qtokf_t[:, h : h + 1],
                        scalar2=0.0,
                        op0=ALU.is_equal,
                        op1=ALU.add,
                        accum_out=pk3[:, h, 0:1],
                    )

                    if DEBUG:
                        cos_sb = scr.tile([128, 512], f32, tag="cossb", name="cos_sb")
                        nc.vector.tensor_copy(
                            out=cos_sb[:], in_=cos[:, 512 * j : 512 * (j + 1)]
                        )
                        nc.sync.dma_start(out=dbg_cos[h], in_=cos_sb[:])

                # Gaussian kernels 1..10: one derf pass over the whole group,
                # then a segmented reduce for the per-chunk sums
                for k in range(1, NK):
                    sim = scr.tile([128, 512 * glen], bf16, tag=f"sim{glen}")
                    nc.scalar.activation(
                        out=sim[:],
                        in_=cos[:],
                        func=AF.Derivative_Erf,
                        scale=SQRT50,
                        bias=derfb_t[:, k : k + 1],
                    )
                    nc.vector.tensor_reduce(
                        out=pk3[:, grp[0] : grp[0] + glen, k : k + 1],
                        in_=sim[:].rearrange("p (c n) -> p c n", c=glen),
                        axis=mybir.AxisListType.X,
                        op=ALU.add,
                    )

            if DEBUG:
                for h in range(DCHUNKS):
                    nc.sync.dma_start(
                        out=dbg_pkq[h], in_=pkq_all[:, NK * h : NK * (h + 1)]
                    )

            # ---------------- tail: clip, log, mask, reduce, dense ----------------
            # batched over all chunks: 2 clips, 2 Ln passes (k0 is a raw count,
            # no 2/sqrt(pi) factor -> own clip+scale), 1 mask-mult, 1 matmul
            pk3 = pkq_all[:].rearrange("p (h k) -> p h k", k=NK)
            nc.vector.tensor_scalar(
                out=pk3[:, :, 0:1], in0=pk3[:, :, 0:1], scalar1=1e-10, scalar2=None,
                op0=ALU.max,
            )
            nc.vector.tensor_scalar(
                out=pk3[:, :, 1:NK], in0=pk3[:, :, 1:NK], scalar1=CLIP, scalar2=None,
                op0=ALU.max,
            )
            # one Ln for all 88 columns (forced after the last derf since it
            # reads the whole pkq tile -> exactly 2 ACT table loads per run);
            # the spurious ln(LN_SCALE) this adds to the k0 columns is undone
            # via the host-folded dense bias (beff)
            lnp = pkpool.tile([128, DCHUNKS * NK], f32, tag="lnpall")
            nc.scalar.activation(
                out=lnp[:], in_=pkq_all[:], func=AF.Ln, scale=LN_SCALE
            )
            nc.vector.tensor_tensor(
                out=lnp[:], in0=lnp[:], in1=qm88_t[:], op=ALU.mult
            )
            pkp = psum1.tile([4, DCHUNKS * NK], f32, tag="pkp")
            nc.tensor.matmul(
                out=pkp[:],
                lhsT=s_sel_t[:],
                rhs=lnp[:],
                start=True,
                stop=True,
            )
            pks = pkpool.tile([4, DCHUNKS * NK], f32, tag="pks")
            nc.vector.tensor_tensor(
                out=pks[:], in0=pkp[:], in1=w88_t[:], op=ALU.mult
            )
            out_acc = pkpool.tile([4, DCHUNKS], f32, tag="outacc")
            pks3 = pks[:].rearrange("p (h k) -> p h k", k=NK)
            for h in range(DCHUNKS):
                nc.vector.reduce_sum(
                    out=out_acc[:, h : h + 1],
                    in_=pks3[:, h],
                    axis=mybir.AxisListType.X,
                )
            nc.vector.tensor_tensor(
                out=out_acc[:], in0=out_acc[:], in1=beff_t[:], op=ALU.add
            )
            nc.sync.dma_start(out=out[:], in_=out_acc[:])

    nc.compile()
    _prog_cache[key] = nc
    return nc


def _wrap16(idx, ncols):
    """[N] int16 -> [128, ncols] with idx i at [i%16, i//16], replicated
    across all eight 16-partition groups (Q7 channel copies)."""
    a = np.asarray(idx, dtype=np.int16).reshape(ncols, 16).T  # [16, ncols]
    return np.tile(a, (8, 1))


def _host_prep(query_tokens, doc_tokens, embed_table, dense_w, dense_b):
    import ml_dtypes

    emb = np.ascontiguousarray(embed_table, dtype=np.float32)
    norms = np.sqrt(np.sum(emb.astype(np.float64) ** 2, axis=1))
    n_emb = emb / np.maximum(norms, 1e-13).astype(np.float32)[:, None]

    qt = np.asarray(query_tokens).astype(np.int64)
    dt = np.asarray(doc_tokens).astype(np.int64)

    s_sel = np.zeros((128, 4), dtype=np.float32)
    for p in range(128):
        s_sel[p, p // 32] = 1.0

    derfb = np.tile(
        (-SQRT50 * np.asarray(MUS, dtype=np.float32)).reshape(1, NK), (128, 1)
    )

    in_maps = []
    for c in range(NCORES):
        qt_c = qt[c * BLOC : (c + 1) * BLOC]  # [32, 20]
        dt_c = dt[c * BLOC : (c + 1) * BLOC]  # [32, 512]
        q_pad = np.zeros((BLOC, QPAD), dtype=np.int64)
        q_pad[:, :Q] = qt_c
        qf = q_pad.reshape(-1)  # [1024] slot order 32b+i
        df = dt_c.reshape(-1)  # [16384]

        uniq = np.unique(np.concatenate([qf, df]))
        assert len(uniq) <= MROWS, len(uniq)
        mtab = np.zeros((MROWS, MSLOT), dtype=ml_dtypes.bfloat16)
        mtab[: len(uniq), :E] = n_emb[uniq].astype(ml_dtypes.bfloat16)
        z = np.searchsorted(uniq, 0)
        if z < len(uniq) and uniq[z] == 0:
            mtab[z, :E] = 0
            mtab[z, BIAS_COL] = MASK_BIAS

        q_i16 = np.searchsorted(uniq, qf).astype(np.int16)
        d_i16 = np.searchsorted(uniq, df).astype(np.int16)

        # one 512-idx gather per (chunk, batch): block (h, beta) covers
        # batch 4h+beta's 512 doc tokens; pre-arranged [128, h*b*32]
        d_idx = (
            np.stack(
                [
                    np.stack(
                        [
                            _wrap16(
                                d_i16[(4 * h + beta) * 512 : (4 * h + beta + 1) * 512],
                                512 // 16,
                            )
                            for beta in range(4)
                        ]
                    )
                    for h in range(DCHUNKS)
                ]
            )  # [h, b, 128, 32]
            .transpose(2, 0, 1, 3)
            .reshape(128, -1)
        )
        d_idx = np.ascontiguousarray(d_idx)
        q_idx = np.stack(
            [_wrap16(q_i16[g * 512 : (g + 1) * 512], 512 // 16) for g in range(2)]
        )

        qtokf = qf.reshape(DCHUNKS, 128).T.astype(np.float32)  # [128, 8]
        qm = (qf > 0).astype(np.float32) * 0.01
        qm001_a = qm.reshape(DCHUNKS, 128).T.astype(np.float32)
        qm88_a = np.repeat(qm001_a, NK, axis=1)  # [128, 88]
        # doc tokens broadcast to [128, h*512]: partition p holds batch p//32
        d_tokf = np.ascontiguousarray(
            np.repeat(
                dt_c.reshape(DCHUNKS, 4, 512).transpose(1, 0, 2), 32, axis=0
            ).reshape(128, -1)
        ).astype(np.float32)

        in_maps.append(
            {
                "mtab": mtab,
                "d_idx": d_idx,
                "q_idx": q_idx,
                "s_sel": s_sel,
                "d_tokf": d_tokf,
                "q_tokf": qtokf,
                "qm88": qm88_a,
                "w88": np.tile(
                    np.asarray(dense_w, dtype=np.float32).reshape(1, NK),
                    (4, DCHUNKS),
                ),
                "beff": (
                    np.asarray(dense_b, np.float32).reshape(-1)[0]
                    + np.asarray(dense_w, np.float32).reshape(-1)[0]
                    * 0.01
                    * LNC
                    * (qt_c > 0).sum(axis=1).reshape(DCHUNKS, 4).T
                ).astype(np.float32),
                "derfb": derfb,
                "qones": np.ones((1, NQTOK), dtype=ml_dtypes.bfloat16),
            }
        )
    return in_maps


def _install_loud_hook():
    # surface exceptions raised inside the PJRT compile callback, which are
    # otherwise swallowed by the C++ layer
    import traceback
    from concourse import bass2jax

    if getattr(bass2jax, "_loud_hook_installed", False):
        return
    orig = bass2jax.neuronx_cc_hook

    def loud(*a, **k):
        try:
            return orig(*a, **k)
        except BaseException:
            traceback.print_exc()
            raise

    bass2jax.neuronx_cc_hook = loud
    bass2jax._loud_hook_installed = True


LAST_RESULTS = None


def kernel(query_tokens, doc_tokens, embed_table, dense_w, dense_b):
    global LAST_RESULTS
    _install_loud_hook()
    from concourse.bass_utils import run_bass_kernel_spmd

    nc = _build_program()
    in_maps = _host_prep(query_tokens, doc_tokens, embed_table, dense_w, dense_b)
    res = run_bass_kernel_spmd(nc, in_maps, list(range(NCORES)))
    LAST_RESULTS = res
    out = np.empty((B,), dtype=np.float32)
    for c in range(NCORES):
        arr = res.results[c]["out"]  # [4, 8]: batch 4h+beta at [beta, h]
        out[c * BLOC : (c + 1) * BLOC] = arr.T.reshape(BLOC)
    return out


# revision 69
# speedup vs baseline: 1.0296x; 1.0296x over previous
"""KNRM kernel for 8 Trainium2 NeuronCores (data-parallel over batch).

Per core (32 batches):
  - host: dedup this core's tokens (~16k unique < int16 max), build a
    pre-normalized bf16 mini-table [17472, 384] (300 emb dims + mask-bias
    column at 300: -1e6 for vocab id 0, else 0), remap token tensors to
    int16 mini-table indices replicated across 16-partition groups (the
    Q7 dma_gather ucode reads a copy per 16-partition channel group).
  - device: per 2048-token chunk, ONE dma_gather(transpose=True) delivers
    embeddings directly in [e, token] layout (partition p, free slot j
    holds element 128j+p), so the cosine matmuls need no PE transposes and
    no PSUM->SBUF copies. Masking is folded into the contraction via the
    bias column (query side forced to 1.0). Gaussian kernel pooling runs
    as ONE scalar-engine pass per kernel using Derivative_Erf
    (d/dx erf = 2/sqrt(pi) * exp(-x^2)) with free-dim accumulation,
    reading cos straight from PSUM; the 2/sqrt(pi) factor is undone by
    the Ln(scale=sqrt(pi)/2) in the tail. k0 (sigma=1e-4, exact token
    match) is a DVE token-equality count scaled by 2/sqrt(pi) to share
    the same tail.
"""

import sys

sys.path.insert(0, "/opt/trn_rl_repo")

import numpy as np

B, Q, D, V, E = 256, 20, 512, 100000, 300
NCORES = 8
BLOC = B // NCORES  # 32 batches per core
QPAD = 32  # query slots per batch (20 real + 12 pad)
NQTOK = BLOC * QPAD  # 1024 query gather slots per core
DCHUNKS = 8  # doc chunks per core
DCTOK = 2048  # doc tokens per chunk (= 4 batches)
NK = 11
MSLOT = 384  # mini-table row elems (bf16) -> 768B, 256B-multiple
MROWS = 17472  # >= max unique tokens per core (16384 doc + 1024 q)
BIAS_COL = 300
MASK_BIAS = -1.0e6

SQRT50 = float(np.sqrt(50.0))
LN_SCALE = float(np.sqrt(np.pi) / 2.0)  # undo derf's 2/sqrt(pi)
K0_SCALE = float(2.0 / np.sqrt(np.pi))
CLIP = 1e-10 / LN_SCALE
# Ln(LN_SCALE*x) applied to the raw k0 count leaves an extra ln(LN_SCALE);
# the correction (-ln(LN_SCALE) per valid q row) is folded into the dense
# bias on the host
LNC = float(-np.log(LN_SCALE))


def _mus(n):
    l = [1.0]
    bs = 2.0 / (n - 1)
    l.append(1 - bs / 2)
    for i in range(1, n - 1):
        l.append(l[i] - bs)
    return l


MUS = _mus(NK)

_prog_cache = {}
DEBUG = False


def _build_program():
    key = ("nc", DEBUG)
    if key in _prog_cache:
        return _prog_cache[key]

    import concourse.bass as bass
    import concourse.bacc as bacc
    import concourse.mybir as mybir
    import concourse.tile as tile
    from concourse import library_config

    f32 = mybir.dt.float32
    bf16 = mybir.dt.bfloat16
    i16 = mybir.dt.int16
    AF = mybir.ActivationFunctionType
    ALU = mybir.AluOpType

    nc = bacc.Bacc(
        "TRN2",
        target_bir_lowering=False,
        debug=False,
        num_devices=NCORES,
        num_swdge_queues=4,
    )

    mtab = nc.dram_tensor("mtab", [MROWS, MSLOT], bf16, kind="ExternalInput").ap()
    d_idx = nc.dram_tensor(
        "d_idx", [128, DCHUNKS * 4 * (512 // 16)], i16, kind="ExternalInput"
    ).ap()
    q_idx = nc.dram_tensor(
        "q_idx", [2, 128, 512 // 16], i16, kind="ExternalInput"
    ).ap()
    s_sel = nc.dram_tensor("s_sel", [128, 4], f32, kind="ExternalInput").ap()
    d_tokf = nc.dram_tensor(
        "d_tokf", [128, DCHUNKS * 512], f32, kind="ExternalInput"
    ).ap()
    q_tokf = nc.dram_tensor("q_tokf", [128, DCHUNKS], f32, kind="ExternalInput").ap()
    qm88 = nc.dram_tensor(
        "qm88", [128, DCHUNKS * NK], f32, kind="ExternalInput"
    ).ap()
    w88 = nc.dram_tensor("w88", [4, DCHUNKS * NK], f32, kind="ExternalInput").ap()
    beff = nc.dram_tensor("beff", [4, DCHUNKS], f32, kind="ExternalInput").ap()

    derfb = nc.dram_tensor("derfb", [128, NK], f32, kind="ExternalInput").ap()
    qones = nc.dram_tensor("qones", [1, NQTOK], bf16, kind="ExternalInput").ap()
    out = nc.dram_tensor("out", [4, DCHUNKS], f32, kind="ExternalOutput").ap()
    dbg_pkq = (
        nc.dram_tensor("dbg_pkq", [DCHUNKS, 128, NK], f32, kind="ExternalOutput").ap()
        if DEBUG
        else None
    )
    dbg_cos = (
        nc.dram_tensor("dbg_cos", [DCHUNKS, 128, 512], f32, kind="ExternalOutput").ap()
        if DEBUG
        else None
    )

    with tile.TileContext(nc) as tc:
        import contextlib

        with contextlib.ExitStack() as ctx:
            const_pool = ctx.enter_context(tc.tile_pool(name="consts", bufs=1))
            qp = ctx.enter_context(tc.tile_pool(name="qprep", bufs=1))
            dpool = ctx.enter_context(tc.tile_pool(name="demb", bufs=3))
            pkpool = ctx.enter_context(tc.tile_pool(name="pk", bufs=1))
            scr = ctx.enter_context(tc.tile_pool(name="scr", bufs=2))
            psum = ctx.enter_context(
                tc.tile_pool(name="psum", bufs=2, space="PSUM")
            )
            psum1 = ctx.enter_context(
                tc.tile_pool(name="psum1", bufs=1, space="PSUM")
            )

            nc.gpsimd.load_library(library_config.mlp)

            # ---------------- Q gather (transposed), 2x512 idxs ----------------
            # queue_num must track the global SWDGE issue index mod 4 so that
            # Tile's round-robin DMA-sem lanes (mod 8) never mix queues: a
            # lane shared by two queues completes out of issue order and the
            # lane's wait threshold passes early (observed as stale dT reads)
            gather_i = 0
            qT = qp.tile([128, 2 * 3 * 512], bf16)
            qT4 = qT[:].rearrange("p (g j n) -> p g j n", g=2, j=3)
            for g in range(2):
                qi = qp.tile([128, 512 // 16], i16, tag=f"qi{g}", name=f"qi{g}")
                nc.sync.dma_start(out=qi[:], in_=q_idx[g])
                nc.gpsimd.dma_gather(
                    out_ap=qT4[:, g],
                    in_ap=mtab[:],
                    idxs_ap=qi[:],
                    num_idxs=512,
                    num_idxs_reg=512,
                    elem_size=MSLOT,
                    transpose=True,
                    queue_num=gather_i % 4,
                )
                gather_i += 1
                # query-side bias multiplier: force e-row 300 (tile 2, part 44)
                nc.sync.dma_start(
                    out=qT4[44:45, g : g + 1, 2, :], in_=qones[:, 512 * g : 512 * (g + 1)]
                )

            # all doc-gather indices in one upfront DMA (host pre-arranged)
            di_all = qp.tile([128, DCHUNKS * 4 * (512 // 16)], i16)
            di4 = di_all[:].rearrange("p (h b n) -> p h b n", h=DCHUNKS, b=4)
            nc.sync.dma_start(out=di_all[:], in_=d_idx[:])

            # derf bias is needed by the first ACT pooling pass: load early
            derfb_t = const_pool.tile([128, NK], f32)
            nc.sync.dma_start(out=derfb_t[:], in_=derfb[:])
            # k0-compare inputs next (the compares run early, off-path, on DVE)
            qtokf_t = const_pool.tile([128, DCHUNKS], f32)
            nc.sync.dma_start(out=qtokf_t[:], in_=q_tokf[:])
            # doc tokens pre-broadcast to all 128 partitions on the host, so
            # the k0 equality check needs no PE broadcast matmul
            dtf_all = qp.tile([128, DCHUNKS * 512], f32)
            dtf3 = dtf_all[:].rearrange("p (h n) -> p h n", h=DCHUNKS)
            nc.sync.dma_start(out=dtf_all[:], in_=d_tokf[:])
            # tail-only consts last
            s_sel_t = const_pool.tile([128, 4], f32)
            nc.sync.dma_start(out=s_sel_t[:], in_=s_sel[:])
            w88_t = const_pool.tile([4, DCHUNKS * NK], f32)
            nc.sync.dma_start(out=w88_t[:], in_=w88[:])
            beff_t = const_pool.tile([4, DCHUNKS], f32)
            nc.sync.dma_start(out=beff_t[:], in_=beff[:])
            qm88_t = const_pool.tile([128, DCHUNKS * NK], f32)
            nc.sync.dma_start(out=qm88_t[:], in_=qm88[:])

            # ---------------- main loop over chunk pairs ----------------
            # all chunks' pooled sums live in one [128, 8*11] tile: chunk h
            # owns columns 11h..11h+11 (k0 at 11h)
            pkq_all = pkpool.tile([128, DCHUNKS * NK], f32, tag="pkqall")
            pk3 = pkq_all[:].rearrange("p (h k) -> p h k", k=NK)
            groups = [(0, 1, 2), (3, 4, 5), (6, 7)]
            for grp in groups:
                glen = len(grp)
                # the group's chunks share one [128, 512*glen] PSUM cos tile
                # so each derf pass covers the whole group; the per-chunk sums
                # come from a segmented DVE reduce afterwards
                cosfull = psum.tile([128, 512 * 3], f32, tag="cos")
                cos = cosfull[:, : 512 * glen]
                for j, h in enumerate(grp):
                    dT = dpool.tile([128, 4 * 3 * 512], bf16, tag="demb")
                    dT4 = dT[:].rearrange("p (b j n) -> p b j n", b=4, j=3)
                    for beta in range(4):
                        nc.gpsimd.dma_gather(
                            out_ap=dT4[:, beta],
                            in_ap=mtab[:],
                            idxs_ap=di4[:, h, beta],
                            num_idxs=512,
                            num_idxs_reg=512,
                            elem_size=MSLOT,
                            transpose=True,
                            queue_num=gather_i % 4,
                        )
                        gather_i += 1



                    for beta in range(4):
                        b_glob = 4 * h + beta
                        g, qs = b_glob // 16, QPAD * (b_glob % 16)
                        cob = cos[32 * beta : 32 * beta + 32, 512 * j : 512 * (j + 1)]
                        nc.tensor.matmul(
                            out=cob,
                            lhsT=qT4[:, g, 0, qs : qs + QPAD],
                            rhs=dT4[:, beta, 0, :],
                            start=True,
                            stop=False,
                            tile_position=(0, 32 * beta),
                        )
                        nc.tensor.matmul(
                            out=cob,
                            lhsT=qT4[:, g, 1, qs : qs + QPAD],
                            rhs=dT4[:, beta, 1, :],
                            start=False,
                            stop=False,
                            tile_position=(0, 32 * beta),
                        )
                        nc.tensor.matmul(
                            out=cob,
                            lhsT=qT4[0:45, g, 2, qs : qs + QPAD],
                            rhs=dT4[0:45, beta, 2, :],
                            start=False,
                            stop=True,
                            tile_position=(0, 32 * beta),
                        )

                    # k0: exact-token-match count
                    cmp = scr.tile([128, 512], f32, tag="cmp")
                    nc.vector.tensor_scalar(
                        out=cmp[:],
                        in0=dtf3[:, h],
                        scalar1=qtokf_t[:, h : h + 1],
                        scalar2=0.0,
                        op0=ALU.is_equal,
                        op1=ALU.add,
                        accum_out=pk3[:, h, 0:1],
                    )

                    if DEBUG:
                        cos_sb = scr.tile([128, 512], f32, tag="cossb", name="cos_sb")
                        nc.vector.tensor_copy(
                            out=cos_sb[:], in_=cos[:, 512 * j : 512 * (j + 1)]
                        )
                        nc.sync.dma_start(out=dbg_cos[h], in_=cos_sb[:])

                # Gaussian kernels 1..10: one derf pass over the whole group,
                # then a segmented reduce for the per-chunk sums
                for k in range(1, NK):
                    sim = scr.tile([128, 512 * glen], bf16, tag=f"sim{glen}")
                    nc.scalar.activation(
                        out=sim[:],
                        in_=cos[:],
                        func=AF.Derivative_Erf,
                        scale=SQRT50,
                        bias=derfb_t[:, k : k + 1],
                    )
                    nc.vector.tensor_reduce(
                        out=pk3[:, grp[0] : grp[0] + glen, k : k + 1],
                        in_=sim[:].rearrange("p (c n) -> p c n", c=glen),
                        axis=mybir.AxisListType.X,
                        op=ALU.add,
                    )

            if DEBUG:
                for h in range(DCHUNKS):
                    nc.sync.dma_start(
                        out=dbg_pkq[h], in_=pkq_all[:, NK * h : NK * (h + 1)]
                    )

            # ---------------- tail: clip, log, mask, reduce, dense ----------------
            # batched over all chunks: 2 clips, 2 Ln passes (k0 is a raw count,
            # no 2/sqrt(pi) factor -> own clip+scale), 1 mask-mult, 1 matmul
            pk3 = pkq_all[:].rearrange("p (h k) -> p h k", k=NK)
            nc.vector.tensor_scalar(
                out=pk3[:, :, 0:1], in0=pk3[:, :, 0:1], scalar1=1e-10, scalar2=None,
                op0=ALU.max,
            )
            nc.vector.tensor_scalar(
                out=pk3[:, :, 1:NK], in0=pk3[:, :, 1:NK], scalar1=CLIP, scalar2=None,
                op0=ALU.max,
            )
            # one Ln for all 88 columns (forced after the last derf since it
            # reads the whole pkq tile -> exactly 2 ACT table loads per run);
            # the spurious ln(LN_SCALE) this adds to the k0 columns is undone
            # via the host-folded dense bias (beff)
            lnp = pkpool.tile([128, DCHUNKS * NK], f32, tag="lnpall")
            nc.scalar.activation(
                out=lnp[:], in_=pkq_all[:], func=AF.Ln, scale=LN_SCALE
            )
            nc.vector.tensor_tensor(
                out=lnp[:], in0=lnp[:], in1=qm88_t[:], op=ALU.mult
            )
            pkp = psum1.tile([4, DCHUNKS * NK], f32, tag="pkp")
            nc.tensor.matmul(
                out=pkp[:],
                lhsT=s_sel_t[:],
                rhs=lnp[:],
                start=True,
                stop=True,
            )
            pks = pkpool.tile([4, DCHUNKS * NK], f32, tag="pks")
            nc.vector.tensor_tensor(
                out=pks[:], in0=pkp[:], in1=w88_t[:], op=ALU.mult
            )
            out_acc = pkpool.tile([4, DCHUNKS], f32, tag="outacc")
            pks3 = pks[:].rearrange("p (h k) -> p h k", k=NK)
            for h in range(DCHUNKS):
                nc.vector.reduce_sum(
                    out=out_acc[:, h : h + 1],
                    in_=pks3[:, h],
                    axis=mybir.AxisListType.X,
                )
            nc.vector.tensor_tensor(
                out=out_acc[:], in0=out_acc[:], in1=beff_t[:], op=ALU.add
            )
            nc.sync.dma_start(out=out[:], in_=out_acc[:])

    nc.compile()
    _prog_cache[key] = nc
    return nc


def _wrap16(idx, ncols):
    """[N] int16 -> [128, ncols] with idx i at [i%16, i//16], replicated
    across all eight 16-partition groups (Q7 channel copies)."""
    a = np.asarray(idx, dtype=np.int16).reshape(ncols, 16).T  # [16, ncols]
    return np.tile(a, (8, 1))


def _host_prep(query_tokens, doc_tokens, embed_table, dense_w, dense_b):
    import ml_dtypes

    emb = np.ascontiguousarray(embed_table, dtype=np.float32)
    norms = np.sqrt(np.sum(emb.astype(np.float64) ** 2, axis=1))
    n_emb = emb / np.maximum(norms, 1e-13).astype(np.float32)[:, None]

    qt = np.asarray(query_tokens).astype(np.int64)
    dt = np.asarray(doc_tokens).astype(np.int64)

    s_sel = np.zeros((128, 4), dtype=np.float32)
    for p in range(128):
        s_sel[p, p // 32] = 1.0

    derfb = np.tile(
        (-SQRT50 * np.asarray(MUS, dtype=np.float32)).reshape(1, NK), (128, 1)
    )

    in_maps = []
    for c in range(NCORES):
        qt_c = qt[c * BLOC : (c + 1) * BLOC]  # [32, 20]
        dt_c = dt[c * BLOC : (c + 1) * BLOC]  # [32, 512]
        q_pad = np.zeros((BLOC, QPAD), dtype=np.int64)
        q_pad[:, :Q] = qt_c
        qf = q_pad.reshape(-1)  # [1024] slot order 32b+i
        df = dt_c.reshape(-1)  # [16384]

        uniq = np.unique(np.concatenate([qf, df]))
        assert len(uniq) <= MROWS, len(uniq)
        mtab = np.zeros((MROWS, MSLOT), dtype=ml_dtypes.bfloat16)
        mtab[: len(uniq), :E] = n_emb[uniq].astype(ml_dtypes.bfloat16)
        z = np.searchsorted(uniq, 0)
        if z < len(uniq) and uniq[z] == 0:
            mtab[z, :E] = 0
            mtab[z, BIAS_COL] = MASK_BIAS

        q_i16 = np.searchsorted(uniq, qf).astype(np.int16)
        d_i16 = np.searchsorted(uniq, df).astype(np.int16)

        # one 512-idx gather per (chunk, batch): block (h, beta) covers
        # batch 4h+beta's 512 doc tokens; pre-arranged [128, h*b*32]
        d_idx = (
            np.stack(
                [
                    np.stack(
                        [
                            _wrap16(
                                d_i16[(4 * h + beta) * 512 : (4 * h + beta + 1) * 512],
                                512 // 16,
                            )
                            for beta in range(4)
                        ]
                    )
                    for h in range(DCHUNKS)
                ]
            )  # [h, b, 128, 32]
            .transpose(2, 0, 1, 3)
            .reshape(128, -1)
        )
        d_idx = np.ascontiguousarray(d_idx)
        q_idx = np.stack(
            [_wrap16(q_i16[g * 512 : (g + 1) * 512], 512 // 16) for g in range(2)]
        )

        qtokf = qf.reshape(DCHUNKS, 128).T.astype(np.float32)  # [128, 8]
        qm = (qf > 0).astype(np.float32) * 0.01
        qm001_a = qm.reshape(DCHUNKS, 128).T.astype(np.float32)
        qm88_a = np.repeat(qm001_a, NK, axis=1)  # [128, 88]
        # doc tokens broadcast to [128, h*512]: partition p holds batch p//32
        d_tokf = np.ascontiguousarray(
            np.repeat(
                dt_c.reshape(DCHUNKS, 4, 512).transpose(1, 0, 2), 32, axis=0
            ).reshape(128, -1)
        ).astype(np.float32)

        in_maps.append(
            {
                "mtab": mtab,
                "d_idx": d_idx,
                "q_idx": q_idx,
                "s_sel": s_sel,
                "d_tokf": d_tokf,
                "q_tokf": qtokf,
                "qm88": qm88_a,
                "w88": np.tile(
                    np.asarray(dense_w, dtype=np.float32).reshape(1, NK),
                    (4, DCHUNKS),
                ),
                "beff": (
                    np.asarray(dense_b, np.float32).reshape(-1)[0]
                    + np.asarray(dense_w, np.float32).reshape(-1)[0]
                    * 0.01
                    * LNC
                    * (qt_c > 0).sum(axis=1).reshape(DCHUNKS, 4).T
                ).astype(np.float32),
                "derfb": derfb,
                "qones": np.ones((1, NQTOK), dtype=ml_dtypes.bfloat16),
            }
        )
    return in_maps


def _install_loud_hook():
    # surface exceptions raised inside the PJRT compile callback, which are
    # otherwise swallowed by the C++ layer
    import traceback
    from concourse import bass2jax

    if getattr(bass2jax, "_loud_hook_installed", False):
        return
    orig = bass2jax.neuronx_cc_hook

    def loud(*a, **k):
        try:
            return orig(*a, **k)
        except BaseException:
            traceback.print_exc()
            raise

    bass2jax.neuronx_cc_hook = loud
    bass2jax._loud_hook_installed = True


LAST_RESULTS = None


def kernel(query_tokens, doc_tokens, embed_table, dense_w, dense_b):
    global LAST_RESULTS
    _install_loud_hook()
    from concourse.bass_utils import run_bass_kernel_spmd

    nc = _build_program()
    in_maps = _host_prep(query_tokens, doc_tokens, embed_table, dense_w, dense_b)
    res = run_bass_kernel_spmd(nc, in_maps, list(range(NCORES)))
    LAST_RESULTS = res
    out = np.empty((B,), dtype=np.float32)
    for c in range(NCORES):
        arr = res.results[c]["out"]  # [4, 8]: batch 4h+beta at [beta, h]
        out[c * BLOC : (c + 1) * BLOC] = arr.T.reshape(BLOC)
    return out
